# revision 18
# baseline (speedup 1.0000x reference)
"""ChebConv (K=4) Trainium2 kernel, v2.

y = sum_k W_k @ T_k(L) x.  Rescaled recurrence (u_k = x_k / 2^(k-1)):
    u1 = L u0, u2 = L u1 - u0/2, u3 = L u2 - u1/4
so every SpMM pass applies the *same* operator (identical one-hot scatter
matrices and edge vals); the subtraction is folded into the PE as one extra
matmul with a constant -c*I stationary; scales (1,1,2,4) fold into W'.

Sharding: 2 batch-groups x 4 V-quarters over 8 cores.  Each core handles 4
batches (256 contiguous bf16 features -> 512-byte gather descriptors, 2x DMA
efficiency vs 256B) and computes the dest-row quarter Q_c of every Chebyshev
iterate.  Full iterates are reassembled between passes with a 4-core DRAM
AllGather.  Pass 3 consumes u3 locally (projection only) -> 2 collectives.

Per window of 128 dest rows: edges sorted by (window, src-half), padded to a
uniform [CWL|CWH] chunk grid of 128 edges; dma_gather fetches source rows
bf16[256]; S[e,r] = val_e * (r == rowloc_e) built in one DVE tensor_scalar
per chunk (all-bf16); PE accumulates S.T @ G into PSUM f32.  Projection via
PE transpose + per-batch matmul with W'_k; y accumulates in an SBUF-resident
bf16 tile, dumped f32 in pass 3.
"""

import sys

for _p in ("/opt/trn_rl_repo",):
    if _p not in sys.path:
        sys.path.insert(0, _p)

import ml_dtypes
import numpy as np

import concourse.bass as bass
import concourse.bacc as bacc
import concourse.mybir as mybir
from concourse import tile
from concourse.bass_utils import run_bass_kernel_spmd

B, FIN, FOUT, V, K, NNZ = 8, 64, 64, 50000, 4, 800000
F = 64
VP = 50176            # 392 windows of 128
HALF = VP // 2
Q = VP // 4           # 12544 rows per quarter
NWQ = 98              # windows per quarter
FW = 256              # 4 batches x 64 features
BF16 = ml_dtypes.bfloat16


def _apply_drain_patch():
    """This walrus build rejects >1 sync waits on the Tile kernel-tail Drain
    (NO_STRUCT codegen path). Emit explicit per-sem EVSEM waits on SP first so
    the drain itself needs none."""
    import bass_rust

    def _patched(self, tick_clock, wait_clock):
        gc = list(tick_clock.global_clock)
        sems = self.sems.allocated()
        for proc, sem in sems.items():
            tick = gc[proc] if proc < len(gc) else 0
            if tick <= 0:
                continue
            name = getattr(sem, "name", "") or ""
            mult = 16 if "DMA" in name else 1
            self.nc.sync.wait_ge(sem, tick * mult)
        self.nc.sync.drain()
        self.nc.all_engine_barrier()
        assert self.sems is not None
        popped = self.nc._tile_sem_poison_stack.pop()
        assert popped is self._sem_poison
        self.nc.clear_and_free_semaphores(list(self.sems.allocated().values()))
        self.nc.all_engine_barrier()

    tile.TileContext._drain_and_barrier = _patched


_apply_drain_patch()


# --------------------------------------------------------------------------
# Host-side edge preprocessing
# --------------------------------------------------------------------------

def preprocess_edges(rows, cols, vals):
    """Per-quarter uniform [window, chunk, lane] grids.

    Returns CWL, CWH and per-quarter eidx [(NGF+1)*128, CW*GWF*8] i16 and
    rv [(NGF+1)*128, GWF*2*CW] bf16 where GWF=4 (tail group has 2 windows,
    padded to full column width)."""
    NWIN = VP // 128
    rows = np.asarray(rows, np.int64)
    cols = np.asarray(cols, np.int64)
    vals = np.asarray(vals, np.float32)

    w = rows // 128
    rloc = (rows % 128).astype(np.float32)
    ishi = (cols >= HALF).astype(np.int64)
    key = w * 2 + ishi
    order = np.argsort(key, kind="stable")
    cnt = np.bincount(key, minlength=NWIN * 2)
    cnt2 = cnt.reshape(NWIN, 2)
    CWL = max(1, int(-(-cnt2[:, 0].max() // 128)))
    CWH = max(1, int(-(-cnt2[:, 1].max() // 128)))
    CW = CWL + CWH

    idx16 = np.zeros((NWIN, CW * 128), np.int16)
    rl = np.zeros((NWIN, CW * 128), np.float32)
    vv = np.zeros((NWIN, CW * 128), np.float32)
    offs = np.concatenate([[0], np.cumsum(cnt)])
    scol = cols[order]
    srl = rloc[order]
    sval = vals[order]
    for wi in range(NWIN):
        nlo = cnt2[wi, 0]
        nhi = cnt2[wi, 1]
        o_lo = offs[wi * 2]
        o_hi = offs[wi * 2 + 1]
        if nlo:
            idx16[wi, :nlo] = scol[o_lo:o_lo + nlo].astype(np.int16)
            rl[wi, :nlo] = srl[o_lo:o_lo + nlo]
            vv[wi, :nlo] = sval[o_lo:o_lo + nlo]
        if nhi:
            h0 = CWL * 128
            idx16[wi, h0:h0 + nhi] = (scol[o_hi:o_hi + nhi] - HALF).astype(np.int16)
            rl[wi, h0:h0 + nhi] = srl[o_hi:o_hi + nhi]
            vv[wi, h0:h0 + nhi] = sval[o_hi:o_hi + nhi]

    GWF = 4
    NGF = NWQ // GWF          # 24 full groups; +1 tail group of 2 windows
    groups = [(g * GWF, GWF) for g in range(NGF)] + [(NGF * GWF, 2)]

    def wrap16(a):
        # [N] flat slots -> [128, N//16] (16-wrap replicated x8)
        n = a.shape[0]
        b = a.reshape(n // 16, 16).T       # [16, N/16]
        return np.tile(b, (8, 1))

    per_quarter = []
    for q in range(4):
        eidx = np.zeros(((NGF + 1) * 128, CW * GWF * 8), np.int16)
        rv = np.zeros(((NGF + 1) * 128, GWF * 2 * CW), np.float32)
        for gi, (w0, gw) in enumerate(groups):
            wins = [q * NWQ + w0 + j for j in range(gw)]
            lo_flat = np.concatenate([idx16[wi, :CWL * 128] for wi in wins])
            hi_flat = np.concatenate([idx16[wi, CWL * 128:] for wi in wins])
            ncol_lo = gw * CWL * 8
            eidx[gi * 128:(gi + 1) * 128, :ncol_lo] = wrap16(lo_flat)
            eidx[gi * 128:(gi + 1) * 128, ncol_lo:ncol_lo + gw * CWH * 8] = (
                wrap16(hi_flat))
            for j, wi in enumerate(wins):
                rl_w = rl[wi].reshape(CW, 128).T       # [128, CW]
                vv_w = vv[wi].reshape(CW, 128).T
                rv[gi * 128:(gi + 1) * 128, j * 2 * CW:j * 2 * CW + CW] = rl_w
                rv[gi * 128:(gi + 1) * 128, j * 2 * CW + CW:(j + 1) * 2 * CW] = vv_w
        per_quarter.append(dict(eidx=eidx, rv=rv))
    return dict(per_quarter=per_quarter, CWL=CWL, CWH=CWH, groups=groups)


# --------------------------------------------------------------------------
# Device program
# --------------------------------------------------------------------------

def build_program(CWL, CWH, UNROLL=6):
    import os
    DBG = int(os.environ.get("KDBG", "9"))
    KNOCC = int(os.environ.get("KNOCC", "0"))
    CW = CWL + CWH
    GWF = 4
    NGF = NWQ // GWF
    f32 = mybir.dt.float32
    bf16 = mybir.dt.bfloat16
    ie = mybir.AluOpType.is_equal
    mu = mybir.AluOpType.mult
    ad = mybir.AluOpType.add
    byp = mybir.AluOpType.bypass

    NQ = 1 if KNOCC else 4
    nc = bacc.Bacc("TRN2", target_bir_lowering=False, num_swdge_queues=NQ)
    u0l = nc.dram_tensor("u0l", [HALF, FW], bf16, kind="ExternalInput")
    u0h = nc.dram_tensor("u0h", [HALF, FW], bf16, kind="ExternalInput")
    u0q = nc.dram_tensor("u0q", [Q, FW], bf16, kind="ExternalInput")
    eidx = nc.dram_tensor("eidx", [(NGF + 1) * 128, CW * GWF * 8],
                          mybir.dt.int16, kind="ExternalInput")
    rv = nc.dram_tensor("rv", [(NGF + 1) * 128, GWF * 2 * CW], f32,
                        kind="ExternalInput")
    wk = nc.dram_tensor("wk", [2 * F, K * F], bf16, kind="ExternalInput")
    iota = nc.dram_tensor("iota", [128, 128], bf16, kind="ExternalInput")
    iden = nc.dram_tensor("iden", [128, 128], bf16, kind="ExternalInput")
    ci2 = nc.dram_tensor("ci2", [128, 128], bf16, kind="ExternalInput")
    ci3 = nc.dram_tensor("ci3", [128, 128], bf16, kind="ExternalInput")
    yout = nc.dram_tensor("yout", [Q, FW], f32, kind="ExternalOutput")

    KDUMP = int(os.environ.get("KDUMP", "0"))
    u1part = nc.dram_tensor("u1part", [Q, FW], bf16)
    u2part = nc.dram_tensor("u2part", [Q, FW], bf16)
    if KDUMP:
        u1d = nc.dram_tensor("u1dump", [Q, FW], bf16, kind="ExternalOutput")
        u2d = nc.dram_tensor("u2dump", [Q, FW], bf16, kind="ExternalOutput")
    u1f = nc.dram_tensor("u1f", [VP, FW], bf16)
    u2f = nc.dram_tensor("u2f", [VP, FW], bf16)

    RG = [[0, 1, 2, 3], [4, 5, 6, 7]]

    def grp(t, g, gw, part=128):
        return t[bass.ds(g * (128 * GWF), 128 * gw), :].rearrange(
            "(j p) f -> p j f", p=part)

    from concourse import library_config

    with tile.TileContext(nc) as tc:
        nc.gpsimd.load_library(library_config.mlp)
        from contextlib import ExitStack
        with ExitStack() as stk:
            cpool = stk.enter_context(tc.tile_pool(name="const", bufs=1))
            ypool = stk.enter_context(tc.tile_pool(name="yacc", bufs=1))
            dpool = stk.enter_context(tc.tile_pool(name="data", bufs=4))
            gpool = stk.enter_context(tc.tile_pool(name="gath", bufs=2))
            spool = stk.enter_context(tc.tile_pool(name="smat", bufs=10))
            xpool = stk.enter_context(tc.tile_pool(name="xtil", bufs=3))
            tpool = stk.enter_context(tc.tile_pool(name="xtra", bufs=6))
            psx_p = stk.enter_context(tc.tile_pool(name="psx", bufs=3, space="PSUM"))
            pst_p = stk.enter_context(tc.tile_pool(name="pst", bufs=3, space="PSUM"))
            psy_p = stk.enter_context(tc.tile_pool(name="psy", bufs=2, space="PSUM"))

            iota_t = cpool.tile([128, 128], bf16)
            nc.sync.dma_start(iota_t[:], iota[:])
            iden_t = cpool.tile([128, 128], bf16)
            nc.sync.dma_start(iden_t[:], iden[:])
            ci2_t = cpool.tile([128, 128], bf16)
            nc.sync.dma_start(ci2_t[:], ci2[:])
            ci3_t = cpool.tile([128, 128], bf16)
            nc.sync.dma_start(ci3_t[:], ci3[:])
            wk_t = cpool.tile([2 * F, K * F], bf16)
            nc.sync.dma_start(wk_t[:], wk[:])
            y_t = ypool.tile([128, NWQ * FW], bf16)
            qbase = [0]

            def make_body(p, gw):
                srcs = {1: (u0l, u0h),
                        2: (u1f[bass.ds(0, HALF), :], u1f[bass.ds(HALF, HALF), :]),
                        3: (u2f[bass.ds(0, HALF), :], u2f[bass.ds(HALF, HALF), :])}[p]
                dst = {1: u1part, 2: u2part, 3: None}[p]
                subt = {1: None, 2: u0q, 3: u1part}[p]
                ci_t = {1: None, 2: ci2_t, 3: ci3_t}[p]

                def body(g):
                    idx_t = dpool.tile([128, GWF * CW * 8], mybir.dt.int16, tag="idx")
                    nc.scalar.dma_start(idx_t[:, 0:gw * CW * 8],
                                        eidx[bass.ds(g * 128, 128), 0:gw * CW * 8])
                    rv_t = dpool.tile([128, GWF * 2 * CW], f32, tag="rv")
                    nc.scalar.dma_start(rv_t[:, 0:gw * 2 * CW],
                                        rv[bass.ds(g * 128, 128), 0:gw * 2 * CW])
                    # SWDGE ring caps one gather at 1024 descriptors; <=8 chunks
                    PIECE = 8
                    g_lo = gpool.tile([128, GWF * CWL, FW], bf16, tag="glo")
                    qbase[0] += 1
                    qn = qbase[0]
                    for c0 in range(0, gw * CWL, PIECE):
                        cn = min(PIECE, gw * CWL - c0)
                        nc.gpsimd.dma_gather(
                            g_lo[:, c0:c0 + cn, :], srcs[0][:],
                            idx_t[:, c0 * 8:(c0 + cn) * 8],
                            cn * 128, cn * 128, FW, queue_num=qn % NQ)
                        qn += 1
                    g_hi = gpool.tile([128, GWF * CWH, FW], bf16, tag="ghi")
                    hoff = gw * CWL * 8
                    for c0 in range(0, gw * CWH, PIECE):
                        cn = min(PIECE, gw * CWH - c0)
                        nc.gpsimd.dma_gather(
                            g_hi[:, c0:c0 + cn, :], srcs[1][:],
                            idx_t[:, hoff + c0 * 8:hoff + (c0 + cn) * 8],
                            cn * 128, cn * 128, FW, queue_num=qn % NQ)
                        qn += 1
                    # u0q (p1: k0 projection; p2: subtract+nothing) / u1part (p3)
                    us_t = None
                    if subt is not None or p == 1:
                        ust = u0q if p in (1, 2) else u1part
                        us_t = dpool.tile([128, GWF, FW], bf16, tag="us")
                        nc.sync.dma_start(us_t[:, 0:gw, :], grp(ust, g, gw))
                    xn_t = xpool.tile([128, GWF, FW], bf16, tag="xn")
                    yo_t = None
                    if p == 3:
                        yo_t = xpool.tile([128, GWF, FW], f32, tag="yo")
                    for j in range(gw):
                        psx = psx_p.tile([128, FW], f32, tag="psx")
                        for c in range(CW):
                            s_t = spool.tile([128, 128], bf16, tag="s")
                            rvo = j * 2 * CW
                            nc.vector.tensor_scalar(
                                s_t[:], iota_t[:],
                                rv_t[:, rvo + c:rvo + c + 1],
                                rv_t[:, rvo + CW + c:rvo + CW + c + 1],
                                ie, mu)
                            if c < CWL:
                                rhs = g_lo[:, j * CWL + c, :]
                            else:
                                rhs = g_hi[:, j * CWH + (c - CWL), :]
                            nc.tensor.matmul(psx[:], s_t[:], rhs,
                                             start=(c == 0),
                                             stop=(p == 1 and c == CW - 1))
                        if ci_t is not None:
                            nc.tensor.matmul(psx[:], ci_t[:], us_t[:, j, :],
                                             start=False, stop=True)
                        nc.scalar.copy(xn_t[:, j, :], psx[:])
                        if DBG < 2:
                            if p == 3:
                                nc.vector.tensor_copy(yo_t[:, j, :], psx[:])
                            continue
                        # per-batch transpose to [fin, v] + project with W'_k
                        psy = psy_p.tile([128, FW], f32, tag="psy")
                        terms = [(0, us_t), (1, xn_t)] if p == 1 else [(p, xn_t)]
                        for b in range(4):
                            for ti, (k, srct) in enumerate(terms):
                                pst = pst_p.tile([64, 128], bf16, tag="pst")
                                nc.tensor.transpose(
                                    pst[:], srct[:, j, b * F:(b + 1) * F],
                                    iden_t[:])
                                xt = tpool.tile([64, 128], bf16, tag="xt")
                                if b % 2 == 0:
                                    nc.vector.tensor_copy(xt[:], pst[:])
                                else:
                                    nc.scalar.copy(xt[:], pst[:])
                                if DBG < 3:
                                    continue
                                nc.tensor.matmul(
                                    psy[:, b * F:(b + 1) * F], xt[:],
                                    wk_t[0:F, k * F:(k + 1) * F],
                                    start=(ti == 0), stop=(ti == len(terms) - 1))
                        if DBG < 4:
                            if p == 3:
                                nc.vector.tensor_copy(yo_t[:, j, :], psx[:])
                            continue
                        w = g * GWF + j
                        yslc = y_t[:, w * FW:(w + 1) * FW]
                        if p == 1:
                            nc.scalar.copy(yslc, psy[:])
                        elif p == 2:
                            nc.vector.tensor_tensor(yslc, yslc, psy[:], ad)
                        else:
                            nc.vector.tensor_tensor(yo_t[:, j, :], yslc, psy[:], ad)
                    if dst is not None:
                        nc.sync.dma_start(grp(dst, g, gw), xn_t[:, 0:gw, :])
                    if p == 3:
                        nc.sync.dma_start(grp(yout, g, gw), yo_t[:, 0:gw, :])

                return body

            for p in (1, 2, 3):
                bf = make_body(p, GWF)
                for g in range(NGF):
                    bf(g)
                make_body(p, 2)(NGF)
                if p == 1 and KNOCC:
                    nc.sync.dma_start(u1f[bass.ds(0, Q), :], u1part[:, :])
                elif p == 1:
                    nc.gpsimd.collective_compute(
                        "AllGather", byp, RG, [u1part[:, :]], [u1f[:, :]])
                    if KDUMP:
                        nc.sync.dma_start(u1d[:, :], u1part[:, :])
                elif p == 2 and KNOCC:
                    nc.sync.dma_start(u2f[bass.ds(0, Q), :], u2part[:, :])
                elif p == 2:
                    nc.gpsimd.collective_compute(
                        "AllGather", byp, RG, [u2part[:, :]], [u2f[:, :]])
                    if KDUMP:
                        nc.sync.dma_start(u2d[:, :], u2part[:, :])

    nc.compile()
    return nc


# --------------------------------------------------------------------------
# Full-size entry point
# --------------------------------------------------------------------------

def _host_inputs(x, L_vals, W, L_rows, L_cols):
    x = np.asarray(x, np.float32)
    W = np.asarray(W, np.float32)
    pre = preprocess_edges(np.asarray(L_rows), np.asarray(L_cols),
                           np.asarray(L_vals, np.float32))
    # W'_k[fin, fo] = scale_k * W[fo, fin*K+k], scale = (1,1,2,4)
    wk_host = np.zeros((FIN, K * FOUT), np.float32)
    Wr = W.reshape(FOUT, FIN, K)
    scale = [1.0, 1.0, 2.0, 4.0]
    for k in range(K):
        wk_host[:, k * FOUT:(k + 1) * FOUT] = scale[k] * Wr[:, :, k].T
    wk_host = np.tile(wk_host, (2, 1)).astype(BF16)
    iota = np.tile(np.arange(128, dtype=np.float32), (128, 1)).astype(BF16)
    iden = np.eye(128, dtype=np.float32).astype(BF16)
    ci2 = (-0.5 * np.eye(128, dtype=np.float32)).astype(BF16)
    ci3 = (-0.25 * np.eye(128, dtype=np.float32)).astype(BF16)

    per_core = []
    for c in range(B):
        bg, q = c // 4, c % 4
        xg = x[bg * 4:(bg + 1) * 4]                      # [4, 64, V]
        x0 = np.zeros((VP, FW), np.float32)
        x0[:V] = xg.transpose(2, 0, 1).reshape(V, FW)
        x0 = x0.astype(BF16)
        pq = pre["per_quarter"][q]
        per_core.append({
            "u0l": np.ascontiguousarray(x0[:HALF]),
            "u0h": np.ascontiguousarray(x0[HALF:]),
            "u0q": np.ascontiguousarray(x0[q * Q:(q + 1) * Q]),
            "eidx": pq["eidx"], "rv": pq["rv"],
            "wk": wk_host, "iota": iota, "iden": iden,
            "ci2": ci2, "ci3": ci3,
        })
    return per_core, pre


_CACHED = {}


def _assemble(res_get):
    out = np.empty((B, FOUT, V), np.float32)
    for c in range(B):
        bg, q = c // 4, c % 4
        yq = np.asarray(res_get(c), np.float32)          # [Q, 256]
        lo = q * Q
        hi = min((q + 1) * Q, V)
        if hi <= lo:
            continue
        blk = yq[:hi - lo].reshape(hi - lo, 4, FOUT)
        for b in range(4):
            out[bg * 4 + b, :, lo:hi] = blk[:, b, :].T
    return out


def kernel(x, L_vals, W, L_rows, L_cols):
    per_core, pre = _host_inputs(x, L_vals, W, L_rows, L_cols)
    key = (pre["CWL"], pre["CWH"])
    if key not in _CACHED:
        _CACHED[key] = build_program(pre["CWL"], pre["CWH"])
    nc = _CACHED[key]
    res = run_bass_kernel_spmd(nc, per_core, list(range(B)))
    return _assemble(lambda c: res.results[c]["yout"])


def bench(x, L_vals, W, L_rows, L_cols, reps=5):
    """Steady-state wall timing of the on-device executable (inputs resident;
    only the donated zero output buffers are re-staged outside the timed span)."""
    import time

    import jax
    from jax.sharding import Mesh, PartitionSpec
    from jax.experimental.shard_map import shard_map
    from concourse import bass2jax

    per_core, pre = _host_inputs(x, L_vals, W, L_rows, L_cols)
    key = (pre["CWL"], pre["CWH"])
    if key not in _CACHED:
        _CACHED[key] = build_program(pre["CWL"], pre["CWH"])
    nc = _CACHED[key]
    bass2jax.install_neuronx_cc_hook()

    import concourse.mybir as _mb
    in_names, out_names, out_avals, zero_outs = [], [], [], []
    for alloc in nc.m.functions[0].allocations:
        if not isinstance(alloc, _mb.MemoryLocationSet):
            continue
        name = alloc.memorylocations[0].name
        pid_name = nc.partition_id_tensor.name if nc.partition_id_tensor else None
        if alloc.kind == "ExternalInput":
            if name != pid_name:
                in_names.append(name)
        elif alloc.kind == "ExternalOutput":
            out_names.append(name)
            shape = tuple(alloc.tensor_shape)
            dtype = _mb.dt.np(alloc.dtype)
            out_avals.append(jax.core.ShapedArray(shape, dtype))
            zero_outs.append(np.zeros(shape, dtype))
    n_params = len(in_names)
    n_outs = len(out_avals)
    all_names = in_names + out_names
    if nc.partition_id_tensor:
        all_names.append(nc.partition_id_tensor.name)

    def _body(*args):
        operands = list(args)
        if nc.partition_id_tensor:
            operands.append(bass2jax.partition_id_tensor())
        outs = bass2jax._bass_exec_p.bind(
            *operands, out_avals=tuple(out_avals), in_names=tuple(all_names),
            out_names=tuple(out_names), lowering_input_output_aliases=(),
            sim_require_finite=True, sim_require_nnan=True, nc=nc)
        return tuple(outs)

    devices = jax.devices()[:B]
    mesh = Mesh(np.asarray(devices), ("core",))
    donate = tuple(range(n_params, n_params + n_outs))
    sharded = jax.jit(
        shard_map(_body, mesh=mesh,
                  in_specs=(PartitionSpec("core"),) * (n_params + n_outs),
                  out_specs=(PartitionSpec("core"),) * n_outs, check_rep=False),
        donate_argnums=donate, keep_unused=True)
    concat_in = [np.concatenate([np.asarray(per_core[c][nm]) for c in range(B)], axis=0)
                 for nm in in_names]
    sh_in = jax.sharding.NamedSharding(mesh, PartitionSpec("core"))
    in_dev = [jax.device_put(a, sh_in) for a in concat_in]
    times = []
    outs = None
    for _ in range(reps):
        zs = [jax.device_put(np.zeros((B * z.shape[0], *z.shape[1:]), z.dtype), sh_in)
              for z in zero_outs]
        jax.block_until_ready(zs)
        t0 = time.perf_counter()
        outs = sharded(*in_dev, *zs)
        jax.block_until_ready(outs)
        times.append(time.perf_counter() - t0)

    chain_times = {}
    for n in (1, 17):
        best = None
        for _ in range(3):
            zsl = [[jax.device_put(
                np.zeros((B * z.shape[0], *z.shape[1:]), z.dtype), sh_in)
                for z in zero_outs] for _ in range(n)]
            jax.block_until_ready(zsl)
            t0 = time.perf_counter()
            outs_l = [sharded(*in_dev, *zs) for zs in zsl]
            jax.block_until_ready(outs_l)
            dt = time.perf_counter() - t0
            best = dt if best is None else min(best, dt)
        chain_times[n] = best
    per_exec = (chain_times[17] - chain_times[1]) / 16.0
    bench.chain_times = chain_times
    bench.per_exec_s = per_exec
    yfull = np.asarray(outs[out_names.index("yout")]).reshape(B, Q, FW)
    return _assemble(lambda c: yfull[c]), times


# revision 20
# speedup vs baseline: 1.9144x; 1.9144x over previous
"""ChebConv (K=4) Trainium2 kernel, v2.

y = sum_k W_k @ T_k(L) x.  Rescaled recurrence (u_k = x_k / 2^(k-1)):
    u1 = L u0, u2 = L u1 - u0/2, u3 = L u2 - u1/4
so every SpMM pass applies the *same* operator (identical one-hot scatter
matrices and edge vals); the subtraction is folded into the PE as one extra
matmul with a constant -c*I stationary; scales (1,1,2,4) fold into W'.

Sharding: 2 batch-groups x 4 V-quarters over 8 cores.  Each core handles 4
batches (256 contiguous bf16 features -> 512-byte gather descriptors, 2x DMA
efficiency vs 256B) and computes the dest-row quarter Q_c of every Chebyshev
iterate.  Full iterates are reassembled between passes with a 4-core DRAM
AllGather.  Pass 3 consumes u3 locally (projection only) -> 2 collectives.

Per window of 128 dest rows: edges sorted by (window, src-half), padded to a
uniform [CWL|CWH] chunk grid of 128 edges; dma_gather fetches source rows
bf16[256]; S[e,r] = val_e * (r == rowloc_e) built in one DVE tensor_scalar
per chunk (all-bf16); PE accumulates S.T @ G into PSUM f32.  Projection via
PE transpose + per-batch matmul with W'_k; y accumulates in an SBUF-resident
bf16 tile, dumped f32 in pass 3.
"""

import sys

for _p in ("/opt/trn_rl_repo",):
    if _p not in sys.path:
        sys.path.insert(0, _p)

import ml_dtypes
import numpy as np

import concourse.bass as bass
import concourse.bacc as bacc
import concourse.mybir as mybir
from concourse import tile
from concourse.bass_utils import run_bass_kernel_spmd

B, FIN, FOUT, V, K, NNZ = 8, 64, 64, 50000, 4, 800000
F = 64
VP = 50176            # 392 windows of 128
HALF = VP // 2
Q = VP // 4           # 12544 rows per quarter
NWQ = 98              # windows per quarter
FW = 256              # 4 batches x 64 features
BF16 = ml_dtypes.bfloat16


def _apply_drain_patch():
    """This walrus build rejects >1 sync waits on the Tile kernel-tail Drain
    (NO_STRUCT codegen path). Emit explicit per-sem EVSEM waits on SP first so
    the drain itself needs none."""
    import bass_rust

    def _patched(self, tick_clock, wait_clock):
        gc = list(tick_clock.global_clock)
        sems = self.sems.allocated()
        for proc, sem in sems.items():
            tick = gc[proc] if proc < len(gc) else 0
            if tick <= 0:
                continue
            name = getattr(sem, "name", "") or ""
            mult = 16 if "DMA" in name else 1
            self.nc.sync.wait_ge(sem, tick * mult)
        self.nc.sync.drain()
        self.nc.all_engine_barrier()
        assert self.sems is not None
        popped = self.nc._tile_sem_poison_stack.pop()
        assert popped is self._sem_poison
        self.nc.clear_and_free_semaphores(list(self.sems.allocated().values()))
        self.nc.all_engine_barrier()

    tile.TileContext._drain_and_barrier = _patched


_apply_drain_patch()


# --------------------------------------------------------------------------
# Host-side edge preprocessing
# --------------------------------------------------------------------------

def preprocess_edges(rows, cols, vals):
    """Per-quarter uniform [window, chunk, lane] grids.

    Returns CWL, CWH and per-quarter eidx [(NGF+1)*128, CW*GWF*8] i16 and
    rv [(NGF+1)*128, GWF*2*CW] bf16 where GWF=4 (tail group has 2 windows,
    padded to full column width)."""
    NWIN = VP // 128
    rows = np.asarray(rows, np.int64)
    cols = np.asarray(cols, np.int64)
    vals = np.asarray(vals, np.float32)

    w = rows // 128
    rloc = (rows % 128).astype(np.float32)
    ishi = (cols >= HALF).astype(np.int64)
    key = w * 2 + ishi
    # secondary sort by source column: ascending gather addresses within
    # each (window, half) bin for HBM locality
    order = np.lexsort((cols, key))
    cnt = np.bincount(key, minlength=NWIN * 2)
    cnt2 = cnt.reshape(NWIN, 2)
    CWL = max(1, int(-(-cnt2[:, 0].max() // 128)))
    CWH = max(1, int(-(-cnt2[:, 1].max() // 128)))
    CW = CWL + CWH

    idx16 = np.zeros((NWIN, CW * 128), np.int16)
    rl = np.zeros((NWIN, CW * 128), np.float32)
    vv = np.zeros((NWIN, CW * 128), np.float32)
    offs = np.concatenate([[0], np.cumsum(cnt)])
    scol = cols[order]
    srl = rloc[order]
    sval = vals[order]
    for wi in range(NWIN):
        nlo = cnt2[wi, 0]
        nhi = cnt2[wi, 1]
        o_lo = offs[wi * 2]
        o_hi = offs[wi * 2 + 1]
        if nlo:
            idx16[wi, :nlo] = scol[o_lo:o_lo + nlo].astype(np.int16)
            rl[wi, :nlo] = srl[o_lo:o_lo + nlo]
            vv[wi, :nlo] = sval[o_lo:o_lo + nlo]
        if nhi:
            h0 = CWL * 128
            idx16[wi, h0:h0 + nhi] = (scol[o_hi:o_hi + nhi] - HALF).astype(np.int16)
            rl[wi, h0:h0 + nhi] = srl[o_hi:o_hi + nhi]
            vv[wi, h0:h0 + nhi] = sval[o_hi:o_hi + nhi]

    GWF = 4
    NGF = NWQ // GWF          # 24 full groups; +1 tail group of 2 windows
    groups = [(g * GWF, GWF) for g in range(NGF)] + [(NGF * GWF, 2)]

    def wrap16(a):
        # [N] flat slots -> [128, N//16] (16-wrap replicated x8)
        n = a.shape[0]
        b = a.reshape(n // 16, 16).T       # [16, N/16]
        return np.tile(b, (8, 1))

    per_quarter = []
    for q in range(4):
        eidx = np.zeros(((NGF + 1) * 128, CW * GWF * 8), np.int16)
        rv = np.zeros(((NGF + 1) * 128, GWF * 2 * CW), np.float32)
        for gi, (w0, gw) in enumerate(groups):
            wins = [q * NWQ + w0 + j for j in range(gw)]
            lo_flat = np.concatenate([idx16[wi, :CWL * 128] for wi in wins])
            hi_flat = np.concatenate([idx16[wi, CWL * 128:] for wi in wins])
            ncol_lo = gw * CWL * 8
            eidx[gi * 128:(gi + 1) * 128, :ncol_lo] = wrap16(lo_flat)
            eidx[gi * 128:(gi + 1) * 128, ncol_lo:ncol_lo + gw * CWH * 8] = (
                wrap16(hi_flat))
            for j, wi in enumerate(wins):
                rl_w = rl[wi].reshape(CW, 128).T       # [128, CW]
                vv_w = vv[wi].reshape(CW, 128).T
                rv[gi * 128:(gi + 1) * 128, j * 2 * CW:j * 2 * CW + CW] = rl_w
                rv[gi * 128:(gi + 1) * 128, j * 2 * CW + CW:(j + 1) * 2 * CW] = vv_w
        per_quarter.append(dict(eidx=eidx, rv=rv))
    return dict(per_quarter=per_quarter, CWL=CWL, CWH=CWH, groups=groups)


# --------------------------------------------------------------------------
# Device program
# --------------------------------------------------------------------------

def build_program(CWL, CWH, UNROLL=6):
    import os
    DBG = int(os.environ.get("KDBG", "9"))
    KNOCC = int(os.environ.get("KNOCC", "0"))
    CW = CWL + CWH
    GWF = 4
    NGF = NWQ // GWF
    f32 = mybir.dt.float32
    bf16 = mybir.dt.bfloat16
    ie = mybir.AluOpType.is_equal
    mu = mybir.AluOpType.mult
    ad = mybir.AluOpType.add
    byp = mybir.AluOpType.bypass

    NQ = 1 if KNOCC else 4
    nc = bacc.Bacc("TRN2", target_bir_lowering=False, num_swdge_queues=NQ)
    u0l = nc.dram_tensor("u0l", [HALF, FW], bf16, kind="ExternalInput")
    u0h = nc.dram_tensor("u0h", [HALF, FW], bf16, kind="ExternalInput")
    u0q = nc.dram_tensor("u0q", [Q, FW], bf16, kind="ExternalInput")
    eidx = nc.dram_tensor("eidx", [(NGF + 1) * 128, CW * GWF * 8],
                          mybir.dt.int16, kind="ExternalInput")
    rv = nc.dram_tensor("rv", [(NGF + 1) * 128, GWF * 2 * CW], f32,
                        kind="ExternalInput")
    wk = nc.dram_tensor("wk", [2 * F, K * F], bf16, kind="ExternalInput")
    iota = nc.dram_tensor("iota", [128, 128], bf16, kind="ExternalInput")
    iden = nc.dram_tensor("iden", [128, 128], bf16, kind="ExternalInput")
    ci2 = nc.dram_tensor("ci2", [128, 128], bf16, kind="ExternalInput")
    ci3 = nc.dram_tensor("ci3", [128, 128], bf16, kind="ExternalInput")
    yout = nc.dram_tensor("yout", [Q, FW], f32, kind="ExternalOutput")

    KDUMP = int(os.environ.get("KDUMP", "0"))
    u1part = nc.dram_tensor("u1part", [Q, FW], bf16)
    u2part = nc.dram_tensor("u2part", [Q, FW], bf16)
    if KDUMP:
        u1d = nc.dram_tensor("u1dump", [Q, FW], bf16, kind="ExternalOutput")
        u2d = nc.dram_tensor("u2dump", [Q, FW], bf16, kind="ExternalOutput")
    u1f = nc.dram_tensor("u1f", [VP, FW], bf16)
    u2f = nc.dram_tensor("u2f", [VP, FW], bf16)

    RG = [[0, 1, 2, 3], [4, 5, 6, 7]]

    def grp(t, g, gw, part=128):
        return t[bass.ds(g * (128 * GWF), 128 * gw), :].rearrange(
            "(j p) f -> p j f", p=part)

    from concourse import library_config

    with tile.TileContext(nc) as tc:
        nc.gpsimd.load_library(library_config.mlp)
        from contextlib import ExitStack
        with ExitStack() as stk:
            cpool = stk.enter_context(tc.tile_pool(name="const", bufs=1))
            ypool = stk.enter_context(tc.tile_pool(name="yacc", bufs=1))
            dpool = stk.enter_context(tc.tile_pool(name="data", bufs=4))
            gpool = stk.enter_context(tc.tile_pool(name="gath", bufs=2))
            spool = stk.enter_context(tc.tile_pool(name="smat", bufs=10))
            xpool = stk.enter_context(tc.tile_pool(name="xtil", bufs=3))
            tpool = stk.enter_context(tc.tile_pool(name="xtra", bufs=6))
            psx_p = stk.enter_context(tc.tile_pool(name="psx", bufs=3, space="PSUM"))
            pst_p = stk.enter_context(tc.tile_pool(name="pst", bufs=3, space="PSUM"))
            psy_p = stk.enter_context(tc.tile_pool(name="psy", bufs=2, space="PSUM"))

            iota_t = cpool.tile([128, 128], bf16)
            nc.sync.dma_start(iota_t[:], iota[:])
            iden_t = cpool.tile([128, 128], bf16)
            nc.sync.dma_start(iden_t[:], iden[:])
            ci2_t = cpool.tile([128, 128], bf16)
            nc.sync.dma_start(ci2_t[:], ci2[:])
            ci3_t = cpool.tile([128, 128], bf16)
            nc.sync.dma_start(ci3_t[:], ci3[:])
            wk_t = cpool.tile([2 * F, K * F], bf16)
            nc.sync.dma_start(wk_t[:], wk[:])
            y_t = ypool.tile([128, NWQ * FW], bf16)
            qbase = [0]

            def make_body(p, gw):
                srcs = {1: (u0l, u0h),
                        2: (u1f[bass.ds(0, HALF), :], u1f[bass.ds(HALF, HALF), :]),
                        3: (u2f[bass.ds(0, HALF), :], u2f[bass.ds(HALF, HALF), :])}[p]
                dst = {1: u1part, 2: u2part, 3: None}[p]
                subt = {1: None, 2: u0q, 3: u1part}[p]
                ci_t = {1: None, 2: ci2_t, 3: ci3_t}[p]

                def body(g):
                    idx_t = dpool.tile([128, GWF * CW * 8], mybir.dt.int16, tag="idx")
                    nc.scalar.dma_start(idx_t[:, 0:gw * CW * 8],
                                        eidx[bass.ds(g * 128, 128), 0:gw * CW * 8])
                    rv_t = dpool.tile([128, GWF * 2 * CW], f32, tag="rv")
                    nc.scalar.dma_start(rv_t[:, 0:gw * 2 * CW],
                                        rv[bass.ds(g * 128, 128), 0:gw * 2 * CW])
                    # SWDGE ring caps one gather at 1024 descriptors; <=8 chunks
                    PIECE = 8
                    g_lo = gpool.tile([128, GWF * CWL, FW], bf16, tag="glo")
                    qbase[0] += 1
                    qn = qbase[0]
                    for c0 in range(0, gw * CWL, PIECE):
                        cn = min(PIECE, gw * CWL - c0)
                        nc.gpsimd.dma_gather(
                            g_lo[:, c0:c0 + cn, :], srcs[0][:],
                            idx_t[:, c0 * 8:(c0 + cn) * 8],
                            cn * 128, cn * 128, FW, queue_num=qn % NQ)
                        qn += 1
                    g_hi = gpool.tile([128, GWF * CWH, FW], bf16, tag="ghi")
                    hoff = gw * CWL * 8
                    for c0 in range(0, gw * CWH, PIECE):
                        cn = min(PIECE, gw * CWH - c0)
                        nc.gpsimd.dma_gather(
                            g_hi[:, c0:c0 + cn, :], srcs[1][:],
                            idx_t[:, hoff + c0 * 8:hoff + (c0 + cn) * 8],
                            cn * 128, cn * 128, FW, queue_num=qn % NQ)
                        qn += 1
                    # u0q (p1: k0 projection; p2: subtract+nothing) / u1part (p3)
                    us_t = None
                    if subt is not None or p == 1:
                        ust = u0q if p in (1, 2) else u1part
                        us_t = dpool.tile([128, GWF, FW], bf16, tag="us")
                        nc.sync.dma_start(us_t[:, 0:gw, :], grp(ust, g, gw))
                    xn_t = xpool.tile([128, GWF, FW], bf16, tag="xn")
                    yo_t = None
                    if p == 3:
                        yo_t = xpool.tile([128, GWF, FW], f32, tag="yo")
                    for j in range(gw):
                        psx = psx_p.tile([128, FW], f32, tag="psx")
                        for c in range(CW):
                            s_t = spool.tile([128, 128], bf16, tag="s")
                            rvo = j * 2 * CW
                            nc.vector.tensor_scalar(
                                s_t[:], iota_t[:],
                                rv_t[:, rvo + c:rvo + c + 1],
                                rv_t[:, rvo + CW + c:rvo + CW + c + 1],
                                ie, mu)
                            if c < CWL:
                                rhs = g_lo[:, j * CWL + c, :]
                            else:
                                rhs = g_hi[:, j * CWH + (c - CWL), :]
                            nc.tensor.matmul(psx[:], s_t[:], rhs,
                                             start=(c == 0),
                                             stop=(p == 1 and c == CW - 1))
                        if ci_t is not None:
                            nc.tensor.matmul(psx[:], ci_t[:], us_t[:, j, :],
                                             start=False, stop=True)
                        nc.scalar.copy(xn_t[:, j, :], psx[:])
                        if DBG < 2:
                            if p == 3:
                                nc.vector.tensor_copy(yo_t[:, j, :], psx[:])
                            continue
                        # per-batch transpose to [fin, v] + project with W'_k
                        psy = psy_p.tile([128, FW], f32, tag="psy")
                        terms = [(0, us_t), (1, xn_t)] if p == 1 else [(p, xn_t)]
                        for b in range(4):
                            for ti, (k, srct) in enumerate(terms):
                                pst = pst_p.tile([64, 128], bf16, tag="pst")
                                nc.tensor.transpose(
                                    pst[:], srct[:, j, b * F:(b + 1) * F],
                                    iden_t[:])
                                xt = tpool.tile([64, 128], bf16, tag="xt")
                                if b % 2 == 0:
                                    nc.vector.tensor_copy(xt[:], pst[:])
                                else:
                                    nc.scalar.copy(xt[:], pst[:])
                                if DBG < 3:
                                    continue
                                nc.tensor.matmul(
                                    psy[:, b * F:(b + 1) * F], xt[:],
                                    wk_t[0:F, k * F:(k + 1) * F],
                                    start=(ti == 0), stop=(ti == len(terms) - 1))
                        if DBG < 4:
                            if p == 3:
                                nc.vector.tensor_copy(yo_t[:, j, :], psx[:])
                            continue
                        w = g * GWF + j
                        yslc = y_t[:, w * FW:(w + 1) * FW]
                        if p == 1:
                            nc.scalar.copy(yslc, psy[:])
                        elif p == 2:
                            nc.vector.tensor_tensor(yslc, yslc, psy[:], ad)
                        else:
                            nc.vector.tensor_tensor(yo_t[:, j, :], yslc, psy[:], ad)
                    if dst is not None:
                        nc.sync.dma_start(grp(dst, g, gw), xn_t[:, 0:gw, :])
                    if p == 3:
                        nc.sync.dma_start(grp(yout, g, gw), yo_t[:, 0:gw, :])

                return body

            for p in (1, 2, 3):
                bf = make_body(p, GWF)
                for g in range(NGF):
                    bf(g)
                make_body(p, 2)(NGF)
                if p == 1 and KNOCC:
                    nc.sync.dma_start(u1f[bass.ds(0, Q), :], u1part[:, :])
                elif p == 1:
                    nc.gpsimd.collective_compute(
                        "AllGather", byp, RG, [u1part[:, :]], [u1f[:, :]])
                    if KDUMP:
                        nc.sync.dma_start(u1d[:, :], u1part[:, :])
                elif p == 2 and KNOCC:
                    nc.sync.dma_start(u2f[bass.ds(0, Q), :], u2part[:, :])
                elif p == 2:
                    nc.gpsimd.collective_compute(
                        "AllGather", byp, RG, [u2part[:, :]], [u2f[:, :]])
                    if KDUMP:
                        nc.sync.dma_start(u2d[:, :], u2part[:, :])

    nc.compile()
    return nc


# --------------------------------------------------------------------------
# Full-size entry point
# --------------------------------------------------------------------------

def _host_inputs(x, L_vals, W, L_rows, L_cols):
    x = np.asarray(x, np.float32)
    W = np.asarray(W, np.float32)
    pre = preprocess_edges(np.asarray(L_rows), np.asarray(L_cols),
                           np.asarray(L_vals, np.float32))
    # W'_k[fin, fo] = scale_k * W[fo, fin*K+k], scale = (1,1,2,4)
    wk_host = np.zeros((FIN, K * FOUT), np.float32)
    Wr = W.reshape(FOUT, FIN, K)
    scale = [1.0, 1.0, 2.0, 4.0]
    for k in range(K):
        wk_host[:, k * FOUT:(k + 1) * FOUT] = scale[k] * Wr[:, :, k].T
    wk_host = np.tile(wk_host, (2, 1)).astype(BF16)
    iota = np.tile(np.arange(128, dtype=np.float32), (128, 1)).astype(BF16)
    iden = np.eye(128, dtype=np.float32).astype(BF16)
    ci2 = (-0.5 * np.eye(128, dtype=np.float32)).astype(BF16)
    ci3 = (-0.25 * np.eye(128, dtype=np.float32)).astype(BF16)

    per_core = []
    for c in range(B):
        bg, q = c // 4, c % 4
        xg = x[bg * 4:(bg + 1) * 4]                      # [4, 64, V]
        x0 = np.zeros((VP, FW), np.float32)
        x0[:V] = xg.transpose(2, 0, 1).reshape(V, FW)
        x0 = x0.astype(BF16)
        pq = pre["per_quarter"][q]
        per_core.append({
            "u0l": np.ascontiguousarray(x0[:HALF]),
            "u0h": np.ascontiguousarray(x0[HALF:]),
            "u0q": np.ascontiguousarray(x0[q * Q:(q + 1) * Q]),
            "eidx": pq["eidx"], "rv": pq["rv"],
            "wk": wk_host, "iota": iota, "iden": iden,
            "ci2": ci2, "ci3": ci3,
        })
    return per_core, pre


_CACHED = {}


def _assemble(res_get):
    out = np.empty((B, FOUT, V), np.float32)
    for c in range(B):
        bg, q = c // 4, c % 4
        yq = np.asarray(res_get(c), np.float32)          # [Q, 256]
        lo = q * Q
        hi = min((q + 1) * Q, V)
        if hi <= lo:
            continue
        blk = yq[:hi - lo].reshape(hi - lo, 4, FOUT)
        for b in range(4):
            out[bg * 4 + b, :, lo:hi] = blk[:, b, :].T
    return out


def kernel(x, L_vals, W, L_rows, L_cols):
    per_core, pre = _host_inputs(x, L_vals, W, L_rows, L_cols)
    key = (pre["CWL"], pre["CWH"])
    if key not in _CACHED:
        _CACHED[key] = build_program(pre["CWL"], pre["CWH"])
    nc = _CACHED[key]
    res = run_bass_kernel_spmd(nc, per_core, list(range(B)))
    return _assemble(lambda c: res.results[c]["yout"])


def bench(x, L_vals, W, L_rows, L_cols, reps=5):
    """Steady-state wall timing of the on-device executable (inputs resident;
    only the donated zero output buffers are re-staged outside the timed span)."""
    import time

    import jax
    from jax.sharding import Mesh, PartitionSpec
    from jax.experimental.shard_map import shard_map
    from concourse import bass2jax

    per_core, pre = _host_inputs(x, L_vals, W, L_rows, L_cols)
    key = (pre["CWL"], pre["CWH"])
    if key not in _CACHED:
        _CACHED[key] = build_program(pre["CWL"], pre["CWH"])
    nc = _CACHED[key]
    bass2jax.install_neuronx_cc_hook()

    import concourse.mybir as _mb
    in_names, out_names, out_avals, zero_outs = [], [], [], []
    for alloc in nc.m.functions[0].allocations:
        if not isinstance(alloc, _mb.MemoryLocationSet):
            continue
        name = alloc.memorylocations[0].name
        pid_name = nc.partition_id_tensor.name if nc.partition_id_tensor else None
        if alloc.kind == "ExternalInput":
            if name != pid_name:
                in_names.append(name)
        elif alloc.kind == "ExternalOutput":
            out_names.append(name)
            shape = tuple(alloc.tensor_shape)
            dtype = _mb.dt.np(alloc.dtype)
            out_avals.append(jax.core.ShapedArray(shape, dtype))
            zero_outs.append(np.zeros(shape, dtype))
    n_params = len(in_names)
    n_outs = len(out_avals)
    all_names = in_names + out_names
    if nc.partition_id_tensor:
        all_names.append(nc.partition_id_tensor.name)

    def _body(*args):
        operands = list(args)
        if nc.partition_id_tensor:
            operands.append(bass2jax.partition_id_tensor())
        outs = bass2jax._bass_exec_p.bind(
            *operands, out_avals=tuple(out_avals), in_names=tuple(all_names),
            out_names=tuple(out_names), lowering_input_output_aliases=(),
            sim_require_finite=True, sim_require_nnan=True, nc=nc)
        return tuple(outs)

    devices = jax.devices()[:B]
    mesh = Mesh(np.asarray(devices), ("core",))
    donate = tuple(range(n_params, n_params + n_outs))
    sharded = jax.jit(
        shard_map(_body, mesh=mesh,
                  in_specs=(PartitionSpec("core"),) * (n_params + n_outs),
                  out_specs=(PartitionSpec("core"),) * n_outs, check_rep=False),
        donate_argnums=donate, keep_unused=True)
    concat_in = [np.concatenate([np.asarray(per_core[c][nm]) for c in range(B)], axis=0)
                 for nm in in_names]
    sh_in = jax.sharding.NamedSharding(mesh, PartitionSpec("core"))
    in_dev = [jax.device_put(a, sh_in) for a in concat_in]
    times = []
    outs = None
    for _ in range(reps):
        zs = [jax.device_put(np.zeros((B * z.shape[0], *z.shape[1:]), z.dtype), sh_in)
              for z in zero_outs]
        jax.block_until_ready(zs)
        t0 = time.perf_counter()
        outs = sharded(*in_dev, *zs)
        jax.block_until_ready(outs)
        times.append(time.perf_counter() - t0)

    chain_times = {}
    for n in (1, 33):
        best = None
        for _ in range(3 if n == 1 else 4):
            zsl = [[jax.device_put(
                np.zeros((B * z.shape[0], *z.shape[1:]), z.dtype), sh_in)
                for z in zero_outs] for _ in range(n)]
            jax.block_until_ready(zsl)
            t0 = time.perf_counter()
            outs_l = [sharded(*in_dev, *zs) for zs in zsl]
            jax.block_until_ready(outs_l)
            dt = time.perf_counter() - t0
            best = dt if best is None else min(best, dt)
        chain_times[n] = best
    per_exec = (chain_times[33] - chain_times[1]) / 32.0
    bench.chain_times = chain_times
    bench.per_exec_s = per_exec
    yfull = np.asarray(outs[out_names.index("yout")]).reshape(B, Q, FW)
    return _assemble(lambda c: yfull[c]), times
